# revision 28
# baseline (speedup 1.0000x reference)
"""Trainium2 Bass kernel for nn_Dynamic_7x7_naivev2 (CSPN-style propagation).

Self-contained: shards the batch across 4 NeuronCores (one full 480-row
sample per core, so no halo rows and no inter-core communication), runs
a Bass/Tile kernel per core, and reassembles the full output.

Wall time is dominated by the host->device tunnel (~40-50 MB/s for
incompressible bytes, with weak LZ-style compression on the wire), so
the wire format is aggressively quantized:
  - guidance ships as int6 mid-tread codes with per-(row, 320-col-chunk)
    f32 scales, packed 4 channels -> 3 bytes in sweep-consumption order
    and unpacked on the DVE (code 32 == 0.0, zero scale rows at the tile
    boundary partitions recreate the zero halo),
  - dynamic ships as u6 codes of sigmoid(dynamic), packed 4 -> 3
    (post-activation quantization shrinks the wire and removes the
    on-device sigmoid),
  - confidence ships as u8 codes,
  - feat_init / feat_fix ship as bf16.
The host->device path is a hand-rolled pjit caller: per-core buffers are
converted with numba quantize/pack loops and streamed non-blocking via a
thread pool while later cores are still converting (the NEFF exec is
sequenced behind the transfers by PJRT), donated output buffers are
zero-created on device, and outputs are fetched shard-parallel.
"""
import copy
import time as _time
from concurrent.futures import ThreadPoolExecutor
from contextlib import ExitStack
from types import SimpleNamespace

import numpy as np

import jax

# Persistent XLA executable cache: the per-call pjit re-compile otherwise
# re-runs the NEFF verify/package step (~0.8 s) on every invocation.
jax.config.update("jax_compilation_cache_dir", "/tmp/jax_cache")
jax.config.update("jax_persistent_cache_min_compile_time_secs", 0)
jax.config.update("jax_persistent_cache_min_entry_size_bytes", 0)

from jax.sharding import Mesh, PartitionSpec, NamedSharding
from jax.experimental.shard_map import shard_map

import bass_rust
import concourse.bass as bass
import concourse.mybir as mybir
from concourse.tile import TileContext

AF = mybir.ActivationFunctionType

NCORES = 4       # one full batch sample per core (no halo rows)
R = 480          # local rows per shard
RPAD = 494       # padded rows for fi: 3 zero + 480 data + 11 zero
W = 640
X = 648          # q/feat tile cols (3 zero margin each side + 2 pad)
NT = 4           # row tiles
TSTEP = 122      # output rows per tile
CH = 48
NDYN = 24
XC = 320         # x chunk width (psum free dim)
CHUNKS = (0, 320)  # output col bases

# (dy, dx) per guidance channel, ring 0 = 3x3 (ch 0:8), 1 = 5x5 (8:24),
# 2 = 7x7 (24:48). Derived numerically from the reference conv.
OFFS = [(1, 1), (1, 0), (1, -1), (0, 1), (0, -1), (-1, 1), (-1, 0), (-1, -1),
        (2, 2), (2, 1), (2, 0), (2, -1), (2, -2), (1, 2), (1, -2), (0, 2),
        (0, -2), (-1, 2), (-1, -2), (-2, 2), (-2, 1), (-2, 0), (-2, -1),
        (-2, -2),
        (3, 3), (3, 2), (3, 1), (3, 0), (3, -1), (3, -2), (3, -3), (2, 3),
        (2, -3), (1, 3), (1, -3), (0, 3), (0, -3), (-1, 3), (-1, -3),
        (-2, 3), (-2, -3), (-3, 3), (-3, 2), (-3, 1), (-3, 0), (-3, -1),
        (-3, -2), (-3, -3)]
RING_RANGES = ((0, 8), (8, 24), (24, 48))


def _consumption_order():
    order = []
    for c0, c1 in RING_RANGES:
        g = {}
        for ch in range(c0, c1):
            dy, dx = OFFS[ch]
            g.setdefault(dy, []).append((ch, dx))
        for dy in sorted(g):
            order.extend(ch for ch, _ in g[dy])
    return order


ORDER = _consumption_order()          # channel index per sweep position
CH_POS = {ch: i for i, ch in enumerate(ORDER)}
NQUAD = CH // 4                       # 12 packed quadruples


def dy_groups(c0, c1):
    """Channels of a ring grouped by row shift dy: [(dy, [(ch, dx), ...])]."""
    g = {}
    for ch in range(c0, c1):
        dy, dx = OFFS[ch]
        g.setdefault(dy, []).append((ch, dx))
    return sorted(g.items())


def tile_geom(t):
    """(base_row, first_valid_part, end_valid_part, q_extent, valid_out)"""
    base = TSTEP * t - 3
    lo = max(0, -base)
    hi = min(128, R - base)
    qhi = min(128, hi + 7)
    vt = min(TSTEP, R - TSTEP * t)
    return base, lo, hi, qhi, vt


def act_recip(nc, out, in_):
    """scalar-engine Reciprocal, bypassing the accuracy guard (we Newton-refine)."""
    eng = nc.scalar
    return eng.add_instruction(
        mybir.InstActivation(
            name=nc.get_next_instruction_name(),
            func=AF.Reciprocal,
            ins=[eng.lower_ap(in_),
                 mybir.ImmediateValue(dtype=mybir.dt.float32, value=0.0),
                 mybir.ImmediateValue(dtype=mybir.dt.float32, value=1.0),
                 mybir.ImmediateValue(dtype=mybir.dt.float32, value=0.0)],
            outs=[eng.lower_ap(out)],
        )
    )


def build_nc(prop_time=6, qdt=mybir.dt.bfloat16, gdt=mybir.dt.float8e3):
    nc = bass.Bass()
    f32 = mybir.dt.float32
    bf16 = mybir.dt.bfloat16
    u8 = mybir.dt.uint8

    # guidance: int6 mid-tread codes (code = clip(round(x/s), -32, 31) + 32),
    # packed 4 channels -> 3 byte planes in sweep-consumption ORDER;
    # scale s per (row, 320-col chunk) in "sc"; dequant = (code - 32) * s.
    ga_in = nc.declare_dram_parameter("ga", [3 * (NQUAD // 2), R, W], u8,
                                      isOutput=False)
    gb_in = nc.declare_dram_parameter("gb2", [3 * (NQUAD // 2), R, W], u8,
                                      isOutput=False)
    # sc cols 0:96 = g scales [row, 2*ch + chunk]; col 96 = ff row scale
    sc_in = nc.declare_dram_parameter("sc", [R, 100], f32, isOutput=False)
    # dyn u6 codes packed 4 -> 3 per iteration group (18 planes) + cf (18)
    # + ff mid-rise codes (19)
    dc_in = nc.declare_dram_parameter("dcp", [3 * (NDYN // 4) + 2, R, W], u8,
                                      isOutput=False)
    CF_PLANE = 3 * (NDYN // 4)
    FF_PLANE = CF_PLANE + 1
    # fi with 3-row zero pad top, 11 pad bottom
    aux_in = nc.declare_dram_parameter("aux", [RPAD, W], bf16,
                                       isOutput=False)
    out = nc.declare_dram_parameter("out", [R, W], bf16, isOutput=True)

    with ExitStack() as ctx:
        tc = ctx.enter_context(TileContext(nc))
        pool = ctx.enter_context(tc.tile_pool(name="main", bufs=1))
        pspool = ctx.enter_context(
            tc.tile_pool(name="ps", bufs=1, space="PSUM"))

        # ---- fixed tiles ----
        # S[dyi][k, j] = 1 iff k == j + dyi — generated on device, no input.
        S = [pool.tile([128, TSTEP], qdt, tag=f"S{i}", name=f"S{i}") for i in range(7)]
        for i in range(7):
            nc.gpsimd.memset(S[i][:], 1.0)
            nc.gpsimd.affine_select(
                out=S[i][:], in_=S[i][:],
                compare_op=mybir.AluOpType.is_equal, fill=0.0,
                base=-i, channel_multiplier=1, pattern=[[-1, TSTEP]])

        ft = [pool.tile([128, X], f32, tag=f"ft{t}", name=f"ft{t}") for t in range(NT)]
        fi_out = [pool.tile([TSTEP, W], f32, tag=f"fio{t}", name=f"fio{t}") for t in range(NT)]
        OM = [pool.tile([TSTEP, W], f32, tag=f"om{t}", name=f"om{t}") for t in range(NT)]
        FF = [pool.tile([TSTEP, W], f32, tag=f"ffp{t}", name=f"ffp{t}") for t in range(NT)]
        A = [[pool.tile([TSTEP, W], f32, tag=f"A{r}{t}", name=f"A{r}{t}") for t in range(NT)]
             for r in range(3)]
        D = [[pool.tile([TSTEP, W], f32, tag=f"D{r}{t}", name=f"D{r}{t}") for t in range(NT)]
             for r in range(3)]

        NQ = 4
        # per-tile unpacked-code buffers (2 rotating quadruple-sets of 4):
        # boundary partitions stay at code 32 from the initial memset (the
        # unpack ops only write each tile's valid rows); code 32 dequants
        # to exactly 0, giving the zero halo for free.
        vset = [[pool.tile([128, W], u8, tag=f"v{t}_{s}", name=f"v{t}_{s}")
                 for s in range(8)] for t in range(NT)]
        # shared packed-byte staging + scratch (only valid rows ever read)
        bset = [pool.tile([128, W], u8, tag=f"bt{s}", name=f"bt{s}")
                for s in range(6)]
        tmpu = [pool.tile([128, W], u8, tag=f"tu{s}", name=f"tu{s}")
                for s in range(4)]
        dynbt = [pool.tile([TSTEP, W], u8, tag=f"dbt{s}", name=f"dbt{s}")
                 for s in range(6)]
        dyntu = [pool.tile([TSTEP, W], u8, tag=f"dtu{s}", name=f"dtu{s}")
                 for s in range(4)]
        # dequantized-g staging and per-tile scale tiles
        NDQ = 3
        dqb = [pool.tile([128, W], qdt, tag=f"dq{i}", name=f"dq{i}")
               for i in range(NDQ)]
        dq_ctr = [0]
        sct = [pool.tile([128, 2 * CH], f32, tag=f"sct{t}", name=f"sct{t}")
               for t in range(NT)]
        qb = [pool.tile([128, X], qdt, tag=f"qb{i}", name=f"qb{i}") for i in range(NQ)]
        fco = [pool.tile([TSTEP, W], f32, tag=f"fco{i}", name=f"fco{i}") for i in range(2)]
        dynb = [pool.tile([TSTEP, 4 * W], u8, tag=f"dynb{i}", name=f"dynb{i}")
                for i in range(2)]
        attb = [pool.tile([TSTEP, 4 * W], f32, tag=f"attb{i}", name=f"attb{i}")
                for i in range(2)]
        cfb = pool.tile([TSTEP, W], u8, tag="cfb", name="cfb")
        cff = pool.tile([TSTEP, W], f32, tag="cff", name="cff")
        ffb = pool.tile([TSTEP, W], u8, tag="ffb", name="ffb")
        ffv = pool.tile([TSTEP, W], bf16, tag="ffv", name="ffv")
        ffs = pool.tile([TSTEP, 1], f32, tag="ffs", name="ffs")
        fstage = pool.tile([128, W], bf16, tag="fstage", name="fstage")
        fstage2 = pool.tile([TSTEP, W], bf16, tag="fstage2", name="fstage2")
        sgn = pool.tile([TSTEP, W], f32, tag="sgn", name="sgn")
        fxb = pool.tile([TSTEP, W], f32, tag="fxb", name="fxb")
        tmp_out = [pool.tile([TSTEP, XC], f32, tag=f"tout{i}", name=f"tout{i}")
                   for i in range(2)]
        NE = 8
        eb = [pool.tile([TSTEP, XC], f32, tag=f"eb{i}", name=f"eb{i}") for i in range(NE)]
        NU = 2
        ub = [pool.tile([128, X], qdt, tag=f"ub{i}", name=f"ub{i}") for i in range(NU)]
        u_ctr = [0]

        for t in range(NT):
            nc.vector.memset(ft[t][:], 0.0)
            nc.vector.memset(sct[t][:], 0.0)
            for s in range(8):
                nc.vector.memset(vset[t][s][:], 32)
        for i in range(NQ):
            nc.vector.memset(qb[i][:], 0.0)

        ALU = mybir.AluOpType

        def unpack_quad(t, qk):
            """Unpack packed byte planes 3qk..3qk+3 (tile t's valid rows)
            into 4 code tiles of vset[t]; returns the 4 tiles."""
            base, _, _, qhi, _ = tile_geom(t)
            p0 = max(0, -base)
            n = min(R, base + qhi) - (base + p0)
            dst4 = vset[t][(qk % 2) * 4:(qk % 2) * 4 + 4]
            bts = bset[(qk % 2) * 3:(qk % 2) * 3 + 3]
            tus = tmpu[(qk % 2) * 2:(qk % 2) * 2 + 2]
            src_t = ga_in if qk < NQUAD // 2 else gb_in
            qkl = qk if qk < NQUAD // 2 else qk - NQUAD // 2
            for j in range(3):
                nc.sync.dma_start(
                    out=bts[j][p0:p0 + n, :],
                    in_=src_t[3 * qkl + j, base + p0:base + p0 + n, :])
            # compute ops must start at partition 0; rows outside the valid
            # window unpack to garbage codes, but their sct scale rows are 0
            # so dequant maps them to exactly 0.
            rs = slice(0, qhi)
            ts = nc.vector.tensor_scalar
            tt = nc.vector.tensor_tensor
            ts(out=dst4[0][rs, :], in0=bts[0][rs, :], scalar1=2, scalar2=None,
               op0=ALU.logical_shift_right)
            ts(out=tus[0][rs, :], in0=bts[0][rs, :], scalar1=3, scalar2=4,
               op0=ALU.bitwise_and, op1=ALU.logical_shift_left)
            ts(out=tus[1][rs, :], in0=bts[1][rs, :], scalar1=4, scalar2=None,
               op0=ALU.logical_shift_right)
            tt(out=dst4[1][rs, :], in0=tus[0][rs, :], in1=tus[1][rs, :],
               op=ALU.bitwise_or)
            ts(out=tus[0][rs, :], in0=bts[1][rs, :], scalar1=15, scalar2=2,
               op0=ALU.bitwise_and, op1=ALU.logical_shift_left)
            ts(out=tus[1][rs, :], in0=bts[2][rs, :], scalar1=6, scalar2=None,
               op0=ALU.logical_shift_right)
            tt(out=dst4[2][rs, :], in0=tus[0][rs, :], in1=tus[1][rs, :],
               op=ALU.bitwise_or)
            ts(out=dst4[3][rs, :], in0=bts[2][rs, :], scalar1=63, scalar2=None,
               op0=ALU.bitwise_and)
            return dst4

        # per-tile scale tiles (sct rows beyond valid stay 0 -> deq 0)
        for t in range(NT):
            base, _, _, qhi, _ = tile_geom(t)
            p0 = max(0, -base)
            n = min(R, base + qhi) - (base + p0)
            nc.sync.dma_start(out=sct[t][p0:p0 + n, :],
                              in_=sc_in[base + p0:base + p0 + n, 0:2 * CH])

        def dequant_g(g, ch, t, qh):
            """(code - 32) * s into a rotating bf16 tile; s per (row, chunk)."""
            dq = dqb[dq_ctr[0] % NDQ]
            dq_ctr[0] += 1
            for ci, cb in enumerate(CHUNKS):
                nc.vector.tensor_scalar(
                    out=dq[0:qh, cb:cb + XC], in0=g[0:qh, cb:cb + XC],
                    scalar1=32.0, scalar2=sct[t][0:qh, 2 * ch + ci:2 * ch + ci + 1],
                    op0=mybir.AluOpType.subtract, op1=mybir.AluOpType.mult)
            return dq

        def psum_tiles():
            return [[pspool.tile([TSTEP, XC], f32, tag=f"ps{r}{c}", name=f"ps{r}{c}")
                     for c in range(2)] for r in range(3)]

        def ring_sweep_grouped(t, ps, prep):
            """48-channel sweep: channels sharing a row shift dy are
            pre-summed (with their column shifts) on the vector engine into
            one u tile, so each ring needs one matmul pair per dy instead
            of per channel."""
            base, lo, hi, qhi, vt = tile_geom(t)
            for ri, (c0, c1) in enumerate(RING_RANGES):
                groups = dy_groups(c0, c1)
                for gi, (dy, members) in enumerate(groups):
                    u = ub[u_ctr[0] % NU]
                    u_ctr[0] += 1
                    for mi, (ch, dx) in enumerate(members):
                        pos = CH_POS[ch]
                        qk = pos // 4
                        if pos % 4 == 0:
                            unpack_quad(t, qk)
                        g = vset[t][(qk % 2) * 4 + pos % 4]
                        qq = qb[ch % NQ]
                        prep(qq, g, qhi, ch)
                        if mi == 0:
                            nc.vector.tensor_copy(
                                out=u[0:qhi, 3:3 + W],
                                in_=qq[0:qhi, 3 + dx:3 + dx + W])
                        else:
                            nc.vector.tensor_add(
                                out=u[0:qhi, 3:3 + W],
                                in0=u[0:qhi, 3:3 + W],
                                in1=qq[0:qhi, 3 + dx:3 + dx + W])
                    first = gi == 0
                    last = gi == len(groups) - 1
                    for ci, cb in enumerate(CHUNKS):
                        nc.tensor.matmul(
                            ps[ri][ci][:],
                            lhsT=S[dy + 3][:],
                            rhs=u[:, cb + 3:cb + 3 + XC],
                            start=first, stop=last)

        # ================= setup =================
        for t in range(NT):
            base, lo, hi, qhi, vt = tile_geom(t)
            n = min(128, R + 3 - TSTEP * t)
            nc.sync.dma_start(out=fstage[0:n, :],
                              in_=aux_in[TSTEP * t:TSTEP * t + n, :])
            nc.vector.tensor_copy(out=ft[t][0:n, 3:3 + W],
                                  in_=fstage[0:n, :])
            r0 = TSTEP * t
            nc.sync.dma_start(out=fstage2[0:vt, :],
                              in_=aux_in[r0 + 3:r0 + 3 + vt, :])
            nc.vector.tensor_copy(out=fi_out[t][0:vt, :],
                                  in_=fstage2[0:vt, :])
            nc.sync.dma_start(out=cfb[0:vt, :],
                              in_=dc_in[CF_PLANE, r0:r0 + vt, :])
            nc.sync.dma_start(out=ffb[0:vt, :],
                              in_=dc_in[FF_PLANE, r0:r0 + vt, :])
            nc.sync.dma_start(out=ffs[0:vt, :],
                              in_=sc_in[r0:r0 + vt, 96:97])
            # ff mid-rise codes -> (code - 127.5) * s_row (sign exact)
            nc.vector.tensor_scalar(
                out=ffv[0:vt, :], in0=ffb[0:vt, :], scalar1=127.5,
                scalar2=ffs[0:vt, :], op0=mybir.AluOpType.subtract,
                op1=mybir.AluOpType.mult)
            # cf codes -> (c + 0.5)/256 in f32
            nc.scalar.activation(out=cff[0:vt], in_=cfb[0:vt], func=AF.Copy,
                                 bias=1.0 / 512, scale=1.0 / 256)
            nc.scalar.sign(out=sgn[0:vt], in_=ffv[0:vt])
            nc.vector.tensor_mul(out=fxb[0:vt, :], in0=sgn[0:vt, :],
                                 in1=cff[0:vt, :])
            nc.scalar.activation(out=OM[t][0:vt], in_=fxb[0:vt], func=AF.Copy,
                                 bias=1.0, scale=-1.0)
            nc.vector.tensor_mul(out=FF[t][0:vt, :], in0=fxb[0:vt, :],
                                 in1=ffv[0:vt, :])

        # aff sums at output rows: A = ring sums of |g|; D = A - sums of g
        # = 2 * ring sums of relu(-g).
        for t in range(NT):
            base, lo, hi, qhi, vt = tile_geom(t)

            def prep_abs(qq, g, qh, ch, t=t):
                dq = dequant_g(g, ch, t, qh)
                nc.scalar.activation(out=qq[0:qh, 3:3 + W], in_=dq[0:qh, :],
                                     func=AF.Abs)

            psA = psum_tiles()
            ring_sweep_grouped(t, psA, prep_abs)
            for ri in range(3):
                for ci, cb in enumerate(CHUNKS):
                    nc.scalar.copy(out=A[ri][t][0:vt, cb:cb + XC],
                                   in_=psA[ri][ci][0:vt, :])

            def prep_negrelu(qq, g, qh, ch, t=t):
                dq = dequant_g(g, ch, t, qh)
                nc.scalar.activation(out=qq[0:qh, 3:3 + W], in_=dq[0:qh, :],
                                     func=AF.Relu, scale=-1.0)

            psB = psum_tiles()
            ring_sweep_grouped(t, psB, prep_negrelu)
            for ri in range(3):
                for ci, cb in enumerate(CHUNKS):
                    nc.scalar.activation(out=D[ri][t][0:vt, cb:cb + XC],
                                         in_=psB[ri][ci][0:vt, :],
                                         func=AF.Copy, scale=2.0)

        # ================= iterations =================
        for it in range(prop_time):
            for t in range(NT):
                base, lo, hi, qhi, vt = tile_geom(t)
                fc = fco[t % 2]
                nc.sync.dma_start(out=fc[0:vt, :],
                                  in_=ft[t][3:3 + vt, 3:3 + W])
                dynt = dynb[t % 2]
                att = attb[t % 2]
                r0 = TSTEP * t
                # unpack this iteration's packed u6 group (planes 3it..3it+3)
                dbt = dynbt[(t % 2) * 3:(t % 2) * 3 + 3]
                for j in range(3):
                    nc.sync.dma_start(out=dbt[j][0:vt, :],
                                      in_=dc_in[3 * it + j, r0:r0 + vt, :])
                ts = nc.vector.tensor_scalar
                tt = nc.vector.tensor_tensor
                du = dyntu[(t % 2) * 2:(t % 2) * 2 + 2]
                dv = slice(0, vt)
                ts(out=dynt[dv, 0 * W:1 * W], in0=dbt[0][dv, :], scalar1=2,
                   scalar2=None, op0=ALU.logical_shift_right)
                ts(out=du[0][dv, :], in0=dbt[0][dv, :], scalar1=3, scalar2=4,
                   op0=ALU.bitwise_and, op1=ALU.logical_shift_left)
                ts(out=du[1][dv, :], in0=dbt[1][dv, :], scalar1=4,
                   scalar2=None, op0=ALU.logical_shift_right)
                tt(out=dynt[dv, 1 * W:2 * W], in0=du[0][dv, :],
                   in1=du[1][dv, :], op=ALU.bitwise_or)
                ts(out=du[0][dv, :], in0=dbt[1][dv, :], scalar1=15, scalar2=2,
                   op0=ALU.bitwise_and, op1=ALU.logical_shift_left)
                ts(out=du[1][dv, :], in0=dbt[2][dv, :], scalar1=6,
                   scalar2=None, op0=ALU.logical_shift_right)
                tt(out=dynt[dv, 2 * W:3 * W], in0=du[0][dv, :],
                   in1=du[1][dv, :], op=ALU.bitwise_or)
                ts(out=dynt[dv, 3 * W:4 * W], in0=dbt[2][dv, :], scalar1=63,
                   scalar2=None, op0=ALU.bitwise_and)
                # u6 codes -> att = (code + 0.5)/64 (sigmoid pre-applied on host)
                nc.scalar.activation(out=att[0:vt, :], in_=dynt[0:vt, :],
                                     func=AF.Copy, bias=1.0 / 128,
                                     scale=1.0 / 64)

                def prep_mul(qq, g, qh, ch, t=t):
                    dq = dequant_g(g, ch, t, qh)
                    nc.vector.tensor_mul(out=qq[0:qh, 3:3 + W],
                                         in0=ft[t][0:qh, 3:3 + W],
                                         in1=dq[0:qh, :])

                ps = psum_tiles()
                ring_sweep_grouped(t, ps, prep_mul)

                for ci, cb in enumerate(CHUNKS):
                    a0 = att[0:vt, 0 * W + cb:0 * W + cb + XC]
                    a1 = att[0:vt, 1 * W + cb:1 * W + cb + XC]
                    a2 = att[0:vt, 2 * W + cb:2 * W + cb + XC]
                    a3 = att[0:vt, 3 * W + cb:3 * W + cb + XC]
                    u0, u1, u2, u3, u4, u5, u6, u7 = (
                        e[0:vt, :] for e in eb)
                    Ac = [A[r][t][0:vt, cb:cb + XC] for r in range(3)]
                    Dc = [D[r][t][0:vt, cb:cb + XC] for r in range(3)]
                    # e = a0*A0 + a1*A1 + a2*A2 + (a3 + 1e-4)
                    nc.vector.tensor_mul(out=u0, in0=a0, in1=Ac[0])
                    nc.vector.tensor_mul(out=u1, in0=a1, in1=Ac[1])
                    nc.vector.tensor_add(out=u0, in0=u0, in1=u1)
                    nc.vector.tensor_mul(out=u2, in0=a2, in1=Ac[2])
                    nc.vector.tensor_scalar_add(u3, a3, 1e-4)
                    nc.vector.tensor_add(out=u2, in0=u2, in1=u3)
                    nc.vector.tensor_add(out=u0, in0=u0, in1=u2)  # u0 = e
                    # d = a0*D0 + a1*D1 + a2*D2 + 1e-4
                    nc.vector.tensor_mul(out=u1, in0=a0, in1=Dc[0])
                    nc.vector.tensor_mul(out=u2, in0=a1, in1=Dc[1])
                    nc.vector.tensor_add(out=u1, in0=u1, in1=u2)
                    nc.vector.tensor_mul(out=u2, in0=a2, in1=Dc[2])
                    nc.vector.tensor_add(out=u1, in0=u1, in1=u2)
                    nc.vector.tensor_scalar_add(u2, u1, 1e-4)  # u2 = d
                    # num = a0*s3 + a1*s5 + a2*s7 + a3*feat + d*feat_init
                    nc.vector.tensor_mul(out=u3, in0=a0,
                                         in1=ps[0][ci][0:vt, :])
                    nc.vector.tensor_mul(out=u4, in0=a1,
                                         in1=ps[1][ci][0:vt, :])
                    nc.vector.tensor_add(out=u3, in0=u3, in1=u4)
                    nc.vector.tensor_mul(out=u4, in0=a2,
                                         in1=ps[2][ci][0:vt, :])
                    fc_c = fc[0:vt, cb:cb + XC]
                    nc.vector.tensor_mul(out=u5, in0=a3, in1=fc_c)
                    nc.vector.tensor_add(out=u4, in0=u4, in1=u5)
                    nc.vector.tensor_mul(out=u5, in0=u2,
                                         in1=fi_out[t][0:vt, cb:cb + XC])
                    nc.vector.tensor_add(out=u3, in0=u3, in1=u4)
                    nc.vector.tensor_add(out=u3, in0=u3, in1=u5)  # num
                    # r = 1/e: ACT table recip + one Newton step
                    act_recip(nc, u6, u0)
                    nc.vector.tensor_mul(out=u4, in0=u0, in1=u6)
                    nc.scalar.activation(out=u4, in_=u4, func=AF.Copy,
                                         bias=2.0, scale=-1.0)
                    nc.vector.tensor_mul(out=u6, in0=u6, in1=u4)
                    nc.vector.tensor_mul(out=u7, in0=u3, in1=u6)
                    to = tmp_out[ci]
                    nc.vector.tensor_mul(out=to[0:vt, :],
                                         in0=OM[t][0:vt, cb:cb + XC],
                                         in1=u7)
                    nc.vector.tensor_add(out=to[0:vt, :],
                                         in0=to[0:vt, :],
                                         in1=FF[t][0:vt, cb:cb + XC])
                    nc.sync.dma_start(
                        out=ft[t][3:3 + vt, 3 + cb:3 + cb + XC],
                        in_=to[0:vt, :])
            # seams between tiles (new feat values)
            for t in range(NT - 1):
                nc.sync.dma_start(out=ft[t + 1][0:3, :],
                                  in_=ft[t][122:125, :])
                nc.sync.dma_start(out=ft[t][125:128, :],
                                  in_=ft[t + 1][3:6, :])

        # ================= output =================
        # bf16 staging halves the D2H bytes (DMA cannot convert dtypes);
        # copy at matching partition offset (DVE cannot shift partitions).
        obuf = [pool.tile([128, W], bf16, tag=f"ob{t}", name=f"ob{t}")
                for t in range(NT)]
        for t in range(NT):
            _, _, _, _, vt = tile_geom(t)
            r0 = TSTEP * t
            nc.vector.tensor_copy(out=obuf[t][:, :],
                                  in_=ft[t][:, 3:3 + W])
            nc.sync.dma_start(out=out[r0:r0 + vt, :],
                              in_=obuf[t][3:3 + vt, :])

    return nc


def fixup_waits(nc, cap=1):
    """Split >cap semaphore waits per instruction into prefix NoOps
    (this toolchain's codegen rejects multi-wait instructions)."""
    n_fixed = 0
    for f in nc.m.functions:
        for bb in f.blocks:
            insts = bb.instructions
            idx = 0
            changed = False
            while idx < len(insts):
                inst = insts[idx]
                si = inst.sync_info
                if si is None or si.on_wait is None or len(si.on_wait) <= cap:
                    idx += 1
                    continue
                waits = list(si.on_wait)
                head = waits[:-cap]
                for j in range(0, len(head), cap):
                    pre = bass_rust.InstNoOp(name=f"{inst.name}_wsplit{j}")
                    pre.engine = inst.engine
                    pre.debug = inst.debug
                    psi = copy.deepcopy(si)
                    psi.on_wait = head[j:j + cap]
                    psi.on_update = []
                    pre.sync_info = psi
                    insts.insert(idx, pre)
                    idx += 1
                si2 = inst.sync_info
                si2.on_wait = waits[-cap:]
                inst.sync_info = si2
                n_fixed += 1
                changed = True
                idx += 1
            if changed:
                bb.instructions = insts
    return n_fixed


# ==================== host-side conversion ====================

_LUT_CACHE = {}
_BUF_CACHE = {}


def _buf(key, shape, dtype):
    b = _BUF_CACHE.get(key)
    if b is None or b.shape != shape or b.dtype != dtype:
        b = _BUF_CACHE[key] = np.empty(shape, dtype)
    return b


def _hi16_vals():
    v = _LUT_CACHE.get("hi16")
    if v is None:
        v = _LUT_CACHE["hi16"] = (
            np.arange(65536, dtype=np.uint32) << np.uint32(16)).view(np.float32)
    return v


def _lut(name, fn):
    """64K-entry u8 LUT over the high 16 bits of an f32."""
    t = _LUT_CACHE.get(name)
    if t is None:
        with np.errstate(invalid="ignore", over="ignore"):
            t = _LUT_CACHE[name] = fn(_hi16_vals())
    return t


def _lut_u6sig():
    return _lut("u6sig", lambda v: np.clip(
        np.floor(64.0 / (1.0 + np.exp(-v.astype(np.float64)))), 0, 63
    ).astype(np.uint8))


def _lut_u8cf():
    return _lut("u8cf", lambda v: np.clip(
        np.floor(v * 256.0), 0, 255).astype(np.uint8))


try:
    import numba

    @numba.njit(cache=False, nogil=True)
    def _lut_gather(src_u16, lut_u8, out_u8):
        for i in range(out_u8.size):
            out_u8[i] = lut_u8[src_u16[i]]

    @numba.njit(cache=False, nogil=True)
    def _quant_pack_g_range(src, order, packed, scales, rowbuf, q0, q1):
        # src [48, R, 640] f32 -> packed [36, R, 640] u8 (quadruples of
        # consumption-ORDER channels, 4 int6 codes -> 3 bytes), scales
        # [R, 96] f32 laid out [row, 2*ch + chunk]. Row-local fused
        # absmax + quantize + pack keeps each row hot in cache.
        nr = packed.shape[1]
        for qk in range(q0, q1):
            for r in range(nr):
                for c in range(4):
                    ch = order[4 * qk + c]
                    for ci in range(2):
                        base = 320 * ci
                        m = 0.0
                        for k in range(320):
                            v = abs(src[ch, r, base + k])
                            if v > m:
                                m = v
                        s = m / 31.49 + 1e-30
                        scales[r, 2 * ch + ci] = s
                        inv = 1.0 / s
                        for k in range(320):
                            q = int(np.floor(src[ch, r, base + k] * inv + 0.5))
                            if q < -32:
                                q = -32
                            elif q > 31:
                                q = 31
                            rowbuf[c, base + k] = q + 32
                for k in range(640):
                    v0 = rowbuf[0, k]
                    v1 = rowbuf[1, k]
                    v2 = rowbuf[2, k]
                    v3 = rowbuf[3, k]
                    pb = 3 * (qk - q0)
                    packed[pb + 0, r, k] = (v0 << 2) | (v1 >> 4)
                    packed[pb + 1, r, k] = ((v1 & 15) << 4) | (v2 >> 2)
                    packed[pb + 2, r, k] = ((v2 & 3) << 6) | v3

    @numba.njit(cache=False, nogil=True)
    def _lut_pack_dyn(src_u16, lut_u8, packed):
        # src_u16 [24, R, 640] (high-16 view of dynamic) -> 6 groups of
        # 4 u6 planes packed into 3 bytes each: packed [18, R, 640] u8.
        nr = packed.shape[1]
        for gk in range(6):
            for r in range(nr):
                for k in range(640):
                    v0 = lut_u8[src_u16[4 * gk + 0, r, k]]
                    v1 = lut_u8[src_u16[4 * gk + 1, r, k]]
                    v2 = lut_u8[src_u16[4 * gk + 2, r, k]]
                    v3 = lut_u8[src_u16[4 * gk + 3, r, k]]
                    packed[3 * gk + 0, r, k] = (v0 << 2) | (v1 >> 4)
                    packed[3 * gk + 1, r, k] = ((v1 & 15) << 4) | (v2 >> 2)
                    packed[3 * gk + 2, r, k] = ((v2 & 3) << 6) | v3
except ImportError:  # pragma: no cover
    _lut_gather = None

    def _pack4(v, packed, base):
        packed[base + 0] = (v[0] << 2) | (v[1] >> 4)
        packed[base + 1] = ((v[1] & 15) << 4) | (v[2] >> 2)
        packed[base + 2] = ((v[2] & 3) << 6) | v[3]

    def _quant_pack_g_range(src, order, packed, scales, rowbuf, q0, q1):
        nr = packed.shape[1]
        x = src.reshape(CH, nr, 2, 320)
        am = np.abs(x).max(axis=-1)                       # [CH, nr, 2]
        s = am / np.float32(31.49) + np.float32(1e-30)
        codes = np.clip(np.floor(x / s[..., None] + 0.5), -32, 31
                        ).astype(np.int16) + 32
        codes = codes.reshape(CH, nr, 640).astype(np.uint8)
        for qk in range(q0, q1):
            chs = [order[4 * qk + c] for c in range(4)]
            for ci in range(2):
                for c in range(4):
                    scales[:, 2 * chs[c] + ci] = s[chs[c], :, ci]
            _pack4([codes[ch] for ch in chs], packed, 3 * (qk - q0))

    def _lut_pack_dyn(src_u16, lut_u8, packed):
        codes = lut_u8[src_u16]
        for gk in range(NDYN // 4):
            _pack4([codes[4 * gk + c] for c in range(4)], packed, 3 * gk)


def _gather_plane(src_f32, lut_u8, dst_u8):
    """Quantize a contiguous f32 plane into u8 codes via the LUT."""
    idx = src_f32.reshape(-1).view(np.uint16)[1::2]
    if _lut_gather is not None:
        _lut_gather(idx, lut_u8, dst_u8.reshape(-1))
    else:
        dst_u8.reshape(-1)[:] = lut_u8[idx]


def _bf16_round(src_f32, dst_u16):
    """f32 -> bf16 bits with round-to-nearest-even-ish (round-half-up)."""
    u = src_f32.reshape(-1).view(np.uint32)
    np.copyto(dst_u16.reshape(-1),
              ((u + np.uint32(0x8000)) >> np.uint32(16)).astype(np.uint16))


# ==================== custom pjit caller ====================

class _Plan:
    """Compiled kernel + jitted sharded executable + reusable buffers."""

    def __init__(self):
        import ml_dtypes
        from concourse import bass2jax
        from concourse.bass2jax import _bass_exec_p, install_neuronx_cc_hook

        nc = build_nc(prop_time=6)
        fixup_waits(nc)
        self.nc = nc
        install_neuronx_cc_hook()

        partition_name = (nc.partition_id_tensor.name
                          if nc.partition_id_tensor else None)
        in_names, out_names, out_avals, zero_outs = [], [], [], []
        for alloc in nc.m.functions[0].allocations:
            if not isinstance(alloc, mybir.MemoryLocationSet):
                continue
            name = alloc.memorylocations[0].name
            if alloc.kind == "ExternalInput":
                if name != partition_name:
                    in_names.append(name)
            elif alloc.kind == "ExternalOutput":
                out_names.append(name)
                shape = tuple(alloc.tensor_shape)
                dtype = mybir.dt.np(alloc.dtype)
                out_avals.append(jax.core.ShapedArray(shape, dtype))
                zero_outs.append(np.zeros(shape, dtype))
        self.in_names = in_names
        self.out_names = out_names
        n_params = len(in_names)
        n_outs = len(out_avals)
        in_names_all = in_names + out_names
        if partition_name is not None:
            in_names_all.append(partition_name)
        donate = tuple(range(n_params, n_params + n_outs))

        def _body(*args):
            operands = list(args)
            if partition_name is not None:
                operands.append(bass2jax.partition_id_tensor())
            outs = _bass_exec_p.bind(
                *operands, out_avals=tuple(out_avals),
                in_names=tuple(in_names_all), out_names=tuple(out_names),
                lowering_input_output_aliases=(),
                sim_require_finite=True, sim_require_nnan=True, nc=nc)
            return tuple(outs)

        self.devices = jax.devices()[:NCORES]
        self.mesh = Mesh(np.asarray(self.devices), ("core",))
        in_specs = (PartitionSpec("core"),) * (n_params + n_outs)
        out_specs = (PartitionSpec("core"),) * len(out_names)
        self.sharded = jax.jit(
            shard_map(_body, mesh=self.mesh, in_specs=in_specs,
                      out_specs=out_specs, check_rep=False),
            donate_argnums=donate, keep_unused=True)
        self.sh = NamedSharding(self.mesh, PartitionSpec("core"))
        self.zero_glob = [
            np.zeros((NCORES * z.shape[0], *z.shape[1:]), z.dtype)
            for z in zero_outs]
        import jax.numpy as jnp
        zshapes = [(tuple(z.shape), z.dtype) for z in self.zero_glob]

        def _mkzeros():
            return tuple(jnp.zeros(s, d) for s, d in zshapes)

        self.zmaker = jax.jit(_mkzeros)
        self.bf = ml_dtypes.bfloat16
        self.pool = ThreadPoolExecutor(max_workers=16)
        self.order = np.asarray(ORDER, np.int64)
        self.rowbuf = np.empty((4, W), np.uint8)
        self.compiled = False

    # ---- per-core host buffers ----
    def core_bufs(self, c):
        return (_buf(("ga", c), (3 * (NQUAD // 2), R, W), np.uint8),
                _buf(("gb2", c), (3 * (NQUAD // 2), R, W), np.uint8),
                _buf(("sc", c), (R, 100), np.float32),
                _buf(("dcp", c), (3 * (NDYN // 4) + 2, R, W), np.uint8),
                _buf(("aux", c), (RPAD, W), np.uint16))

    def convert_core(self, c, guidance, dynamic, confidence, feat_init,
                     feat_fix):
        b = c
        ga, gb2, sc, dcp, aux = self.core_bufs(c)
        # cheapest buffers first so the wire starts streaming immediately
        aux[0:3] = 0
        _bf16_round(feat_init[b, 0], aux[3:3 + R])
        aux[3 + R:RPAD] = 0
        yield ((4, aux.view(self.bf)),)
        dyn_u16 = dynamic[b].view(np.uint16)[..., 1::2]
        _lut_pack_dyn(dyn_u16, _lut_u6sig(), dcp[:3 * (NDYN // 4)])
        _gather_plane(confidence[b, 0], _lut_u8cf(), dcp[3 * (NDYN // 4)])
        # ff -> u8 mid-rise codes with per-row scale (col 96 of sc);
        # mid-rise has no zero code, so sign(ff) is exact on device
        ffp = feat_fix[b, 0]
        am = np.abs(ffp).max(axis=1)
        s = am / np.float32(127.49) + np.float32(1e-30)
        codes = np.clip(np.floor(ffp / s[:, None]), -128, 127)
        dcp[3 * (NDYN // 4) + 1] = (codes + 128).astype(np.uint8)
        sc[:, 96] = s
        sc[:, 97:] = 0
        yield ((3, dcp),)
        # guidance -> packed int6 mid-tread codes + per-(row, chunk) scales,
        # in two halves so the first half streams while the second converts
        _quant_pack_g_range(guidance[b], self.order, ga, sc, self.rowbuf,
                            0, NQUAD // 2)
        yield ((0, ga),)
        _quant_pack_g_range(guidance[b], self.order, gb2, sc, self.rowbuf,
                            NQUAD // 2, NQUAD)
        yield ((1, gb2), (2, sc))

    def put_core(self, c, bufs):
        d = self.devices[c]
        return [self.pool.submit(self._put_one, a, d) for a in bufs]

    @staticmethod
    def _put_one(a, d):
        # no block_until_ready: PJRT sequences the exec after the transfer
        # on-device, and fetching outputs at the end of run() is the fence
        # that makes host-buffer reuse safe across calls.
        return jax.device_put(a, d)

    def _make_zeros(self):
        try:
            zs = self.zmaker()
            return [jax.device_put(z, self.sh) if z.sharding != self.sh else z
                    for z in zs]
        except Exception:
            return [jax.device_put(z, self.sh) for z in self.zero_glob]

    def run(self, guidance, dynamic, confidence, feat_init, feat_fix):
        # dispatch the on-device zero creation first so its round trip
        # overlaps the conversion + streaming below
        glob_z = self._make_zeros()
        futs = [[None] * 5 for _ in range(NCORES)]
        for c in range(NCORES):
            d = self.devices[c]
            for batch in self.convert_core(c, guidance, dynamic, confidence,
                                           feat_init, feat_fix):
                for i, a in batch:
                    futs[c][i] = self.pool.submit(self._put_one, a, d)
        n_in = len(self.in_names)
        glob_in = []
        for i in range(n_in):
            shards = [futs[c][i].result() for c in range(NCORES)]
            gshape = (NCORES * shards[0].shape[0], *shards[0].shape[1:])
            glob_in.append(jax.make_array_from_single_device_arrays(
                gshape, self.sh, shards))
        outs = self.sharded(*glob_in, *glob_z)
        self.compiled = True
        # threaded per-shard fetch
        out0 = outs[0]
        shard_arrs = list(self.pool.map(
            lambda s: np.asarray(s.data),
            sorted(out0.addressable_shards,
                   key=lambda s: s.index[0].start or 0)))
        return shard_arrs


_PLAN = [None]


def _get_plan():
    if _PLAN[0] is None:
        _PLAN[0] = _Plan()
    return _PLAN[0]


def _assemble(shards):
    """NCORES per-core [480, W] bf16 shards -> [4, 1, 480, 640] f32."""
    outf = np.empty((4, 1, 480, W), np.float32)
    ou = outf.view(np.uint32)
    for c in range(NCORES):
        s16 = shards[c].view(np.uint16).astype(np.uint32)
        ou[c, 0] = s16 << 16
    return outf


def kernel(feat_init, guidance, dynamic, confidence, feat_fix,
           _trace=False):
    guidance = np.ascontiguousarray(np.asarray(guidance, np.float32))
    dynamic = np.ascontiguousarray(np.asarray(dynamic, np.float32))
    confidence = np.ascontiguousarray(np.asarray(confidence, np.float32))
    feat_init = np.ascontiguousarray(np.asarray(feat_init, np.float32))
    feat_fix = np.ascontiguousarray(np.asarray(feat_fix, np.float32))
    plan = _get_plan()
    if _trace:
        # trace path goes through the stock spmd runner so test.py can pull
        # an NTFF profile when the axon hook supports it
        from concourse.bass_utils import run_bass_kernel_spmd
        in_maps = []
        for c in range(NCORES):
            bufs = [None] * 5
            for batch in plan.convert_core(c, guidance, dynamic, confidence,
                                           feat_init, feat_fix):
                for i, a in batch:
                    bufs[i] = a
            in_maps.append(dict(zip(plan.in_names, bufs)))
        try:
            res = run_bass_kernel_spmd(plan.nc, in_maps,
                                       core_ids=list(range(NCORES)),
                                       trace=True)
            shards = [res.results[c]["out"] for c in range(NCORES)]
            return _assemble(shards), res
        except Exception:
            shards = plan.run(guidance, dynamic, confidence, feat_init,
                              feat_fix)
            res = SimpleNamespace(exec_time_ns=None, mean_exec_time_ns=None,
                                  max_exec_time_core_id=None,
                                  instructions_and_trace=None)
            return _assemble(shards), res
    try:
        shards = plan.run(guidance, dynamic, confidence, feat_init, feat_fix)
    except Exception:
        # transient NRT_EXEC_UNIT_UNRECOVERABLE device wedges have been
        # observed on this fabric; one retry usually clears them
        _time.sleep(2.0)
        shards = plan.run(guidance, dynamic, confidence, feat_init, feat_fix)
    return _assemble(shards)
